# revision 8
# baseline (speedup 1.0000x reference)
"""AdditiveAttention fused Trainium2 kernel.

Computes, for vision_features (B, DV), ts_features (B, DT) with B=1024,
DV=2048, DT=A=512:

    vp = vision @ Wv_w.T + Wv_b                     (B, A)
    tp = ts @ Wt_w.T + Wt_b                         (B, A)
    scores[b,i] = sum_j v_w[j]*tanh(tp[b,i]+vp[b,j])   (+ v_b, dropped:
                                                     softmax shift-invariant)
    attn = softmax(scores, axis=1)
    out = concat([vision, ts * attn], axis=1)       (B, DV+DT)

Data parallel over 8 NeuronCores: each core owns 128 batch rows and the
replicated (small) weights.  Per core the B*A*A tanh tensor is built in
SBUF chunks:

  GPSIMD  partition_broadcast of the (tp+Wt_b) row  -> Bt [128, 512]
  DVE     tensor_scalar add of vp^T column (bias per j-partition)
          -> S [128, CH*2048] (j on partitions, (batch, i) on free dim)
  ACT     one big tanh per CH batches (the bottleneck: B*A*A/128 lanes)
  PE      scores[b,:] += v_col[jblk].T @ G[jblk]  (K=128 contraction over j)

then a row softmax (free-dim) on the PSUM scores and the elementwise
epilogue.
"""

import numpy as np

import concourse.bacc as bacc
import concourse.tile as tile
import concourse.mybir as mybir
from concourse import masks
from concourse.bass_utils import run_bass_kernel_spmd

N_CORES = 8
B, DV, DT, A = 1024, 2048, 512, 512
NB = B // N_CORES          # batch rows per core (== 128 partitions)
P = 128
ND = DV // P               # 16 d-chunks for the vision projection
NA = A // P                # 4 a/j blocks
CH = 2                     # batches fused per tanh ACT op (FD = CH*2048)

F32 = mybir.dt.float32
AF = mybir.ActivationFunctionType
ALU = mybir.AluOpType
AX = mybir.AxisListType


def build():
    nc = bacc.Bacc(
        "TRN2", target_bir_lowering=False, debug=False, num_devices=N_CORES
    )
    vis_d = nc.dram_tensor("vision_features", [NB, DV], F32, kind="ExternalInput").ap()
    ts_d = nc.dram_tensor("ts_features", [NB, DT], F32, kind="ExternalInput").ap()
    wv_d = nc.dram_tensor("Wv_w", [A, DV], F32, kind="ExternalInput").ap()
    wvb_d = nc.dram_tensor("Wv_b", [A], F32, kind="ExternalInput").ap()
    wt_d = nc.dram_tensor("Wt_w", [A, DT], F32, kind="ExternalInput").ap()
    wtb_d = nc.dram_tensor("Wt_b", [A], F32, kind="ExternalInput").ap()
    vw_d = nc.dram_tensor("v_w", [A], F32, kind="ExternalInput").ap()
    out_d = nc.dram_tensor("out", [NB, DV + DT], F32, kind="ExternalOutput").ap()

    with tile.TileContext(nc) as tc:
        with (
            tc.tile_pool(name="persist", bufs=1) as persist,
            tc.tile_pool(name="scores", bufs=1, space="PSUM") as scores_pool,
        ):
            vis = persist.tile([P, DV], F32, tag="vis")
            ts_sb = persist.tile([P, DT], F32, tag="ts")
            tpb = persist.tile([P, A], F32, tag="tpb")     # tp + Wt_b, [b, i]
            vpT = persist.tile([P, A], F32, tag="vpT")     # [a%128, ablk*128+b]
            vcol = persist.tile([P, NA], F32, tag="vcol")  # v_w column per jblk
            # sliding-window one-hot weights: vwin[jb] is zeros except
            # column 31 = v_w[jb-block].  lhsT = vwin[jb][:, 31-r:63-r]
            # is a [128, 32] one-hot-at-r stationary operand, so the
            # matmul adds v.G into row r of a 32-aligned PSUM strip and
            # exact zeros into the other 31 rows (PE out partition base
            # must be 32-aligned, so M=1 at partition b is illegal).
            vwin = [
                persist.tile([P, 64], F32, tag=f"vwin{jb}", name=f"vwin{jb}")
                for jb in range(NA)
            ]
            wvbc = persist.tile([P, NA], F32, tag="wvbc")  # Wv_b column per ablk
            wtbr = persist.tile([1, A], F32, tag="wtbr")
            wtb_bc = persist.tile([P, A], F32, tag="wtb_bc")
            ident = persist.tile([P, P], F32, tag="ident")

            masks.make_identity(nc, ident[:])
            nc.sync.dma_start(vis[:], vis_d[:])
            nc.sync.dma_start(ts_sb[:], ts_d[:])
            nc.sync.dma_start(vcol[:], vw_d.rearrange("(blk p) -> p blk", p=P))
            nc.sync.dma_start(wvbc[:], wvb_d.rearrange("(blk p) -> p blk", p=P))
            nc.sync.dma_start(wtbr[0:1, :], wtb_d[:])
            nc.gpsimd.partition_broadcast(wtb_bc[:], wtbr[0:1, :])
            for jb in range(NA):
                nc.vector.memset(vwin[jb][:], 0.0)
                nc.vector.tensor_copy(vwin[jb][:, 31:32], vcol[:, jb:jb + 1])

            scores_ps = scores_pool.tile([P, A], F32, tag="scores")

            # ---- stage 1: projections (vp^T and tp) --------------------
            with (
                tc.tile_pool(name="s1w", bufs=1) as s1w,
                tc.tile_pool(name="s1t", bufs=1) as s1t,
                tc.tile_pool(name="s1ps", bufs=4, space="PSUM") as s1ps,
                tc.tile_pool(name="s1acc", bufs=1, space="PSUM") as s1acc,
            ):
                # vision side: vp^T[a, b] = sum_d Wv[a, d] * vis[b, d]
                wv = []
                for ablk in range(NA):
                    t = s1w.tile([P, DV], F32, tag=f"wv{ablk}")
                    nc.sync.dma_start(t[:], wv_d[ablk * P:(ablk + 1) * P, :])
                    wv.append(t)
                visT = []
                wvT = []
                for dc in range(ND):
                    tps = s1ps.tile([P, P], F32, tag="tr_ps")
                    nc.tensor.transpose(tps[:], vis[:, dc * P:(dc + 1) * P], ident[:])
                    vt = s1t.tile([P, P], F32, tag=f"visT{dc}")
                    nc.vector.tensor_copy(vt[:], tps[:])
                    visT.append(vt)
                    wt_tile = s1t.tile([P, A], F32, tag=f"wvT{dc}")
                    for ablk in range(NA):
                        tps2 = s1ps.tile([P, P], F32, tag="tr_ps")
                        nc.tensor.transpose(
                            tps2[:], wv[ablk][:, dc * P:(dc + 1) * P], ident[:]
                        )
                        nc.vector.tensor_copy(
                            wt_tile[:, ablk * P:(ablk + 1) * P], tps2[:]
                        )
                    wvT.append(wt_tile)
                vpT_ps = s1acc.tile([P, A], F32, tag="vpT_ps")
                for ablk in range(NA):
                    for dc in range(ND):
                        nc.tensor.matmul(
                            vpT_ps[:, ablk * P:(ablk + 1) * P],
                            wvT[dc][:, ablk * P:(ablk + 1) * P],
                            visT[dc],
                            start=(dc == 0),
                            stop=(dc == ND - 1),
                        )
                for ablk in range(NA):
                    nc.vector.tensor_scalar_add(
                        vpT[:, ablk * P:(ablk + 1) * P],
                        vpT_ps[:, ablk * P:(ablk + 1) * P],
                        wvbc[:, ablk:ablk + 1],
                    )

                # ts side: tp[b, a] = sum_d ts[b, d] * Wt[a, d]
                wt = []
                for ablk in range(NA):
                    t = s1w.tile([P, DT], F32, tag=f"wt{ablk}")
                    nc.sync.dma_start(t[:], wt_d[ablk * P:(ablk + 1) * P, :])
                    wt.append(t)
                tsT = []
                wtT = []
                for dc in range(DT // P):
                    tps = s1ps.tile([P, P], F32, tag="tr_ps")
                    nc.tensor.transpose(tps[:], ts_sb[:, dc * P:(dc + 1) * P], ident[:])
                    tt = s1t.tile([P, P], F32, tag=f"tsT{dc}")
                    nc.vector.tensor_copy(tt[:], tps[:])
                    tsT.append(tt)
                    wtt = s1t.tile([P, A], F32, tag=f"wtT{dc}")
                    for ablk in range(NA):
                        tps2 = s1ps.tile([P, P], F32, tag="tr_ps")
                        nc.tensor.transpose(
                            tps2[:], wt[ablk][:, dc * P:(dc + 1) * P], ident[:]
                        )
                        nc.vector.tensor_copy(
                            wtt[:, ablk * P:(ablk + 1) * P], tps2[:]
                        )
                    wtT.append(wtt)
                tp_ps = s1acc.tile([P, A], F32, tag="tp_ps")
                for dc in range(DT // P):
                    nc.tensor.matmul(
                        tp_ps[:],
                        tsT[dc],
                        wtT[dc],
                        start=(dc == 0),
                        stop=(dc == DT // P - 1),
                    )
                nc.vector.tensor_add(tpb[:], tp_ps[:], wtb_bc[:])

            # ---- stage 2: the hot loop ---------------------------------
            # Compute engines can only address SBUF starting at partition
            # 0/32/64/96, so the per-batch (tp+Wt_b) row is staged to
            # partition 0 with a tiny DMA, then partition-broadcast by the
            # (otherwise idle) GPSIMD.
            with tc.tile_pool(name="hot", bufs=3) as hot:
                for b2 in range(NB // CH):
                    b0 = b2 * CH
                    rowstage = hot.tile([1, CH * A], F32, tag="rowstage")
                    nc.sync.dma_start(rowstage[0:1, :], tpb[b0:b0 + CH, :])
                    bt = hot.tile([P, CH * A], F32, tag="Bt")
                    nc.gpsimd.partition_broadcast(bt[:], rowstage[0:1, :])
                    S = hot.tile([P, CH * NA * A], F32, tag="S")
                    for ci in range(CH):
                        b = b0 + ci
                        for jb in range(NA):
                            o = ci * NA * A + jb * A
                            nc.vector.tensor_scalar_add(
                                S[:, o:o + A],
                                bt[:, ci * A:(ci + 1) * A],
                                vpT[:, jb * P + b:jb * P + b + 1],
                            )
                    G = hot.tile([P, CH * NA * A], F32, tag="G")
                    nc.scalar.activation(G[:], S[:], AF.Tanh)
                    for ci in range(CH):
                        b = b2 * CH + ci
                        strip, r = divmod(b, 32)
                        for jb in range(NA):
                            o = ci * NA * A + jb * A
                            nc.tensor.matmul(
                                scores_ps[strip * 32:(strip + 1) * 32, :],
                                vwin[jb][:, 31 - r:63 - r],
                                G[:, o:o + A],
                                start=(r == 0 and jb == 0),
                                stop=(r == 31 and jb == NA - 1),
                                tile_position=(0, strip * 32),
                            )

            # ---- stage 3: softmax + epilogue ---------------------------
            with tc.tile_pool(name="epi", bufs=1) as epi:
                neg_m = epi.tile([P, 1], F32, tag="neg_m")
                nc.vector.tensor_reduce(
                    neg_m[:], scores_ps[:], axis=AX.X, op=ALU.max, negate=True
                )
                ex = epi.tile([P, A], F32, tag="ex")
                ssum = epi.tile([P, 1], F32, tag="ssum")
                nc.scalar.activation(
                    ex[:], scores_ps[:], AF.Exp, bias=neg_m[:], accum_out=ssum[:]
                )
                rcp = epi.tile([P, 1], F32, tag="rcp")
                nc.vector.reciprocal(rcp[:], ssum[:])
                attw = epi.tile([P, A], F32, tag="attw")
                nc.vector.tensor_scalar_mul(attw[:], ex[:], rcp[:])
                att = epi.tile([P, A], F32, tag="att")
                nc.vector.tensor_mul(att[:], attw[:], ts_sb[:])
                nc.sync.dma_start(out_d[:, 0:DV], vis[:])
                nc.sync.dma_start(out_d[:, DV:DV + DT], att[:])

    nc.compile()
    return nc


_NC_CACHE = None


def _get_nc():
    global _NC_CACHE
    if _NC_CACHE is None:
        _NC_CACHE = build()
    return _NC_CACHE


def make_in_maps(vision_features, ts_features, Wv_w, Wv_b, Wt_w, Wt_b, v_w):
    shared = {
        "Wv_w": np.ascontiguousarray(Wv_w, dtype=np.float32),
        "Wv_b": np.ascontiguousarray(Wv_b, dtype=np.float32),
        "Wt_w": np.ascontiguousarray(Wt_w, dtype=np.float32),
        "Wt_b": np.ascontiguousarray(Wt_b, dtype=np.float32),
        "v_w": np.ascontiguousarray(v_w, dtype=np.float32),
    }
    in_maps = []
    for c in range(N_CORES):
        sl = slice(c * NB, (c + 1) * NB)
        in_maps.append(
            {
                "vision_features": np.ascontiguousarray(
                    vision_features[sl], dtype=np.float32
                ),
                "ts_features": np.ascontiguousarray(
                    ts_features[sl], dtype=np.float32
                ),
                **shared,
            }
        )
    return in_maps


def kernel(
    vision_features, ts_features, Wv_w, Wv_b, Wt_w, Wt_b, v_w, v_b=None, **_unused
):
    # v_b shifts every score of a row equally; softmax is invariant to it.
    nc = _get_nc()
    in_maps = make_in_maps(
        vision_features, ts_features, Wv_w, Wv_b, Wt_w, Wt_b, v_w
    )
    res = run_bass_kernel_spmd(nc, in_maps, core_ids=list(range(N_CORES)))
    return np.concatenate([res.results[c]["out"] for c in range(N_CORES)], axis=0)


# revision 15
# speedup vs baseline: 1.7029x; 1.7029x over previous
"""AdditiveAttention fused Trainium2 kernel.

Computes, for vision_features (B, DV), ts_features (B, DT) with B=1024,
DV=2048, DT=A=512:

    vp = vision @ Wv_w.T + Wv_b                     (B, A)
    tp = ts @ Wt_w.T + Wt_b                         (B, A)
    scores[b,i] = sum_j v_w[j]*tanh(tp[b,i]+vp[b,j])   (+ v_b, dropped:
                                                     softmax shift-invariant)
    attn = softmax(scores, axis=1)
    out = concat([vision, ts * attn], axis=1)       (B, DV+DT)

Data parallel over 8 NeuronCores: each core owns 128 batch rows and the
replicated (small) weights.  Per core the B*A*A tanh tensor is built in
SBUF chunks:

  GPSIMD  partition_broadcast of the (tp+Wt_b) row  -> Bt [128, 512]
  DVE     tensor_scalar add of vp^T column (bias per j-partition)
          -> S [128, CH*2048] (j on partitions, (batch, i) on free dim)
  ACT     one big tanh per CH batches (the bottleneck: B*A*A/128 lanes)
  PE      scores[b,:] += v_col[jblk].T @ G[jblk]  (K=128 contraction over j)

then a row softmax (free-dim) on the PSUM scores and the elementwise
epilogue.
"""

import numpy as np

import concourse.bacc as bacc
import concourse.tile as tile
import concourse.mybir as mybir
from concourse import masks
from concourse.bass_utils import run_bass_kernel_spmd

N_CORES = 8
B, DV, DT, A = 1024, 2048, 512, 512
NB = B // N_CORES          # batch rows per core (== 128 partitions)
P = 128
ND = DV // P               # 16 d-chunks for the vision projection
NA = A // P                # 4 a/j blocks
CH = 2                     # batches fused per tanh ACT op (FD = CH*2048)

F32 = mybir.dt.float32
F32R = mybir.dt.float32r
AF = mybir.ActivationFunctionType
ALU = mybir.AluOpType
AX = mybir.AxisListType


def build():
    nc = bacc.Bacc(
        "TRN2", target_bir_lowering=False, debug=False, num_devices=N_CORES
    )
    vis_d = nc.dram_tensor("vision_features", [NB, DV], F32, kind="ExternalInput").ap()
    ts_d = nc.dram_tensor("ts_features", [NB, DT], F32, kind="ExternalInput").ap()
    wv_d = nc.dram_tensor("Wv_w", [A, DV], F32, kind="ExternalInput").ap()
    wvb_d = nc.dram_tensor("Wv_b", [A], F32, kind="ExternalInput").ap()
    wt_d = nc.dram_tensor("Wt_w", [A, DT], F32, kind="ExternalInput").ap()
    wtb_d = nc.dram_tensor("Wt_b", [A], F32, kind="ExternalInput").ap()
    vw_d = nc.dram_tensor("v_w", [A], F32, kind="ExternalInput").ap()
    out_d = nc.dram_tensor("out", [NB, DV + DT], F32, kind="ExternalOutput").ap()

    with tile.TileContext(nc) as tc:
        with (
            tc.tile_pool(name="persist", bufs=1) as persist,
            tc.tile_pool(name="scores", bufs=1, space="PSUM") as scores_pool,
        ):
            vis = persist.tile([P, DV], F32, tag="vis")
            ts_sb = persist.tile([P, DT], F32, tag="ts")
            tpb = persist.tile([P, A], F32, tag="tpb")     # tp + Wt_b, [b, i]
            vpT = persist.tile([P, A], F32, tag="vpT")     # [a%128, ablk*128+b]
            vcol = persist.tile([P, NA], F32, tag="vcol")  # v_w column per jblk
            # sliding-window one-hot weights: vwin[jb] is zeros except
            # column 63 = v_w[jb-block].  lhsT = vwin[jb][:, 63-r:127-r]
            # is a [128, 64] one-hot-at-r stationary operand, so the
            # matmul adds v.G into row r of a 64-row PSUM tile and exact
            # zeros into the other rows.  (fp32r matmuls may only write
            # PSUM starting at partition 0, and the PE out base must be
            # 32-aligned anyway, so M=1 at partition b is illegal; the
            # two 64-batch halves go to separate PSUM tiles and are
            # realigned into one [128, 512] SBUF tile by DMA afterwards.)
            vwin = [
                persist.tile([P, 2 * 64], F32R, tag=f"vwin{jb}", name=f"vwin{jb}")
                for jb in range(NA)
            ]
            zwin = persist.tile([P, 2 * 64], F32, tag="zwin")
            wvbc = persist.tile([P, NA], F32, tag="wvbc")  # Wv_b column per ablk
            wtbr = persist.tile([1, A], F32, tag="wtbr")
            wtb_bc = persist.tile([P, A], F32, tag="wtb_bc")
            ident = persist.tile([P, P], F32, tag="ident")

            masks.make_identity(nc, ident[:])
            nc.sync.dma_start(vis[:], vis_d[:])
            nc.sync.dma_start(ts_sb[:], ts_d[:])
            nc.sync.dma_start(vcol[:], vw_d.rearrange("(blk p) -> p blk", p=P))
            nc.sync.dma_start(wvbc[:], wvb_d.rearrange("(blk p) -> p blk", p=P))
            nc.sync.dma_start(wtbr[0:1, :], wtb_d[:])
            nc.gpsimd.partition_broadcast(wtb_bc[:], wtbr[0:1, :])
            nc.vector.memset(zwin[:], 0.0)
            for jb in range(NA):
                nc.vector.tensor_copy(vwin[jb][:], zwin[:])
                nc.vector.tensor_copy(vwin[jb][:, 63:64], vcol[:, jb:jb + 1])

            scores_ps = [
                scores_pool.tile([P, A], F32, tag=f"scores{t}", name=f"scores{t}")
                for t in range(2)
            ]

            # ---- stage 1: projections (vp^T and tp) --------------------
            with (
                tc.tile_pool(name="s1w", bufs=1) as s1w,
                tc.tile_pool(name="s1t", bufs=1) as s1t,
                tc.tile_pool(name="s1ps", bufs=4, space="PSUM") as s1ps,
                tc.tile_pool(name="s1acc", bufs=1, space="PSUM") as s1acc,
            ):
                # vision side: vp^T[a, b] = sum_d Wv[a, d] * vis[b, d]
                wv = []
                for ablk in range(NA):
                    t = s1w.tile([P, DV], F32, tag=f"wv{ablk}")
                    nc.sync.dma_start(t[:], wv_d[ablk * P:(ablk + 1) * P, :])
                    wv.append(t)
                visT = []
                wvT = []
                for dc in range(ND):
                    tps = s1ps.tile([P, P], F32, tag="tr_ps")
                    nc.tensor.transpose(tps[:], vis[:, dc * P:(dc + 1) * P], ident[:])
                    vt = s1t.tile([P, P], F32, tag=f"visT{dc}")
                    nc.vector.tensor_copy(vt[:], tps[:])
                    visT.append(vt)
                    wt_tile = s1t.tile([P, A], F32, tag=f"wvT{dc}")
                    for ablk in range(NA):
                        tps2 = s1ps.tile([P, P], F32, tag="tr_ps")
                        nc.tensor.transpose(
                            tps2[:], wv[ablk][:, dc * P:(dc + 1) * P], ident[:]
                        )
                        nc.vector.tensor_copy(
                            wt_tile[:, ablk * P:(ablk + 1) * P], tps2[:]
                        )
                    wvT.append(wt_tile)
                vpT_ps = s1acc.tile([P, A], F32, tag="vpT_ps")
                for ablk in range(NA):
                    for dc in range(ND):
                        nc.tensor.matmul(
                            vpT_ps[:, ablk * P:(ablk + 1) * P],
                            wvT[dc][:, ablk * P:(ablk + 1) * P],
                            visT[dc],
                            start=(dc == 0),
                            stop=(dc == ND - 1),
                        )
                for ablk in range(NA):
                    nc.vector.tensor_scalar_add(
                        vpT[:, ablk * P:(ablk + 1) * P],
                        vpT_ps[:, ablk * P:(ablk + 1) * P],
                        wvbc[:, ablk:ablk + 1],
                    )

                # ts side: tp[b, a] = sum_d ts[b, d] * Wt[a, d]
                wt = []
                for ablk in range(NA):
                    t = s1w.tile([P, DT], F32, tag=f"wt{ablk}")
                    nc.sync.dma_start(t[:], wt_d[ablk * P:(ablk + 1) * P, :])
                    wt.append(t)
                tsT = []
                wtT = []
                for dc in range(DT // P):
                    tps = s1ps.tile([P, P], F32, tag="tr_ps")
                    nc.tensor.transpose(tps[:], ts_sb[:, dc * P:(dc + 1) * P], ident[:])
                    tt = s1t.tile([P, P], F32, tag=f"tsT{dc}")
                    nc.vector.tensor_copy(tt[:], tps[:])
                    tsT.append(tt)
                    wtt = s1t.tile([P, A], F32, tag=f"wtT{dc}")
                    for ablk in range(NA):
                        tps2 = s1ps.tile([P, P], F32, tag="tr_ps")
                        nc.tensor.transpose(
                            tps2[:], wt[ablk][:, dc * P:(dc + 1) * P], ident[:]
                        )
                        nc.vector.tensor_copy(
                            wtt[:, ablk * P:(ablk + 1) * P], tps2[:]
                        )
                    wtT.append(wtt)
                tp_ps = s1acc.tile([P, A], F32, tag="tp_ps")
                for dc in range(DT // P):
                    nc.tensor.matmul(
                        tp_ps[:],
                        tsT[dc],
                        wtT[dc],
                        start=(dc == 0),
                        stop=(dc == DT // P - 1),
                    )
                nc.vector.tensor_add(tpb[:], tp_ps[:], wtb_bc[:])

            # ---- stage 2: the hot loop ---------------------------------
            # Compute engines can only address SBUF starting at partition
            # 0/32/64/96, so the per-batch (tp+Wt_b) row is staged to
            # partition 0 with a tiny DMA, then partition-broadcast by the
            # (otherwise idle) GPSIMD.
            with tc.tile_pool(name="hot", bufs=3) as hot:
                for b2 in range(NB // CH):
                    b0 = b2 * CH
                    rowstage = hot.tile([1, CH * A], F32, tag="rowstage")
                    nc.sync.dma_start(rowstage[0:1, :], tpb[b0:b0 + CH, :])
                    bt = hot.tile([P, CH * A], F32, tag="Bt")
                    nc.gpsimd.partition_broadcast(bt[:], rowstage[0:1, :])
                    S = hot.tile([P, CH * NA * A], F32, tag="S")
                    for ci in range(CH):
                        b = b0 + ci
                        for jb in range(NA):
                            o = ci * NA * A + jb * A
                            nc.vector.tensor_scalar_add(
                                S[:, o:o + A],
                                bt[:, ci * A:(ci + 1) * A],
                                vpT[:, jb * P + b:jb * P + b + 1],
                            )
                    G = hot.tile([P, CH * NA * A], F32R, tag="G")
                    nc.scalar.activation(G[:], S[:], AF.Tanh)
                    for ci in range(CH):
                        b = b2 * CH + ci
                        t, r = divmod(b, 64)
                        for jb in range(NA):
                            o = ci * NA * A + jb * A
                            nc.tensor.matmul(
                                scores_ps[t][0:64, :],
                                vwin[jb][:, 63 - r:127 - r],
                                G[:, o:o + A],
                                start=(r == 0 and jb == 0),
                                stop=(r == 63 and jb == NA - 1),
                            )

            # ---- stage 3: softmax + epilogue ---------------------------
            with tc.tile_pool(name="epi", bufs=1) as epi:
                scores_sb = epi.tile([P, A], F32, tag="scores_sb")
                shalf = epi.tile([P, A], F32, tag="shalf")
                nc.vector.tensor_copy(scores_sb[0:64, :], scores_ps[0][0:64, :])
                nc.vector.tensor_copy(shalf[0:64, :], scores_ps[1][0:64, :])
                nc.sync.dma_start(scores_sb[64:128, :], shalf[0:64, :])
                neg_m = epi.tile([P, 1], F32, tag="neg_m")
                nc.vector.tensor_reduce(
                    neg_m[:], scores_sb[:], axis=AX.X, op=ALU.max, negate=True
                )
                ex = epi.tile([P, A], F32, tag="ex")
                ssum = epi.tile([P, 1], F32, tag="ssum")
                nc.scalar.activation(
                    ex[:], scores_sb[:], AF.Exp, bias=neg_m[:], accum_out=ssum[:]
                )
                rcp = epi.tile([P, 1], F32, tag="rcp")
                nc.vector.reciprocal(rcp[:], ssum[:])
                attw = epi.tile([P, A], F32, tag="attw")
                nc.vector.tensor_scalar_mul(attw[:], ex[:], rcp[:])
                att = epi.tile([P, A], F32, tag="att")
                nc.vector.tensor_mul(att[:], attw[:], ts_sb[:])
                nc.sync.dma_start(out_d[:, 0:DV], vis[:])
                nc.sync.dma_start(out_d[:, DV:DV + DT], att[:])

    nc.compile()
    return nc


_NC_CACHE = None


def _get_nc():
    global _NC_CACHE
    if _NC_CACHE is None:
        _NC_CACHE = build()
    return _NC_CACHE


def make_in_maps(vision_features, ts_features, Wv_w, Wv_b, Wt_w, Wt_b, v_w):
    shared = {
        "Wv_w": np.ascontiguousarray(Wv_w, dtype=np.float32),
        "Wv_b": np.ascontiguousarray(Wv_b, dtype=np.float32),
        "Wt_w": np.ascontiguousarray(Wt_w, dtype=np.float32),
        "Wt_b": np.ascontiguousarray(Wt_b, dtype=np.float32),
        "v_w": np.ascontiguousarray(v_w, dtype=np.float32),
    }
    in_maps = []
    for c in range(N_CORES):
        sl = slice(c * NB, (c + 1) * NB)
        in_maps.append(
            {
                "vision_features": np.ascontiguousarray(
                    vision_features[sl], dtype=np.float32
                ),
                "ts_features": np.ascontiguousarray(
                    ts_features[sl], dtype=np.float32
                ),
                **shared,
            }
        )
    return in_maps


def kernel(
    vision_features, ts_features, Wv_w, Wv_b, Wt_w, Wt_b, v_w, v_b=None, **_unused
):
    # v_b shifts every score of a row equally; softmax is invariant to it.
    nc = _get_nc()
    in_maps = make_in_maps(
        vision_features, ts_features, Wv_w, Wv_b, Wt_w, Wt_b, v_w
    )
    res = run_bass_kernel_spmd(nc, in_maps, core_ids=list(range(N_CORES)))
    return np.concatenate([res.results[c]["out"] for c in range(N_CORES)], axis=0)


# revision 30
# speedup vs baseline: 1.9858x; 1.1662x over previous
"""AdditiveAttention fused Trainium2 kernel.

Computes, for vision_features (B, DV), ts_features (B, DT) with B=1024,
DV=2048, DT=A=512:

    vp = vision @ Wv_w.T + Wv_b                     (B, A)
    tp = ts @ Wt_w.T + Wt_b                         (B, A)
    scores[b,i] = sum_j v_w[j]*tanh(tp[b,i]+vp[b,j])   (+ v_b, dropped:
                                                     softmax shift-invariant)
    attn = softmax(scores, axis=1)
    out = concat([vision, ts * attn], axis=1)       (B, DV+DT)

Data parallel over 8 NeuronCores: each core owns 128 batch rows and the
replicated (small) weights.  Per core, the B*A*A tanh tensor (the
reference's 1 GB intermediate) is built and consumed in SBUF chunks:

  DMA     stages the per-batch (tp+Wt_b) rows to partition 0 (compute
          engines can only address SBUF from partitions 0/32/64/96)
  GPSIMD  partition_broadcast of those rows           -> Bt [128, CHP*512]
  DVE     tensor_scalar add of the vp^T column (bias per j-partition)
          -> S (j on partitions, (batch, i) on the free dim)
  ACT     one FD=4096 tanh per 4-batch group — the bottleneck:
          B*A*A / 128 lanes / 1.2 GHz  ~=  218 us/core floor
  PE      fp32r matmuls against a sliding one-hot v_w window accumulate
          scores rows into two 64-row PSUM tiles (fp32r matmul output
          must start at PSUM partition 0)

The batch sweep runs in two j-phases (j-blocks {0,1} then {2,3}) so the
first tanh only needs the left half of Wv; the right half streams in
during phase 0.  Weights are pre-transposed/pre-chunked on the host so
each weight block is a single fully contiguous DMA.  Softmax is done per
64-row half directly on the PSUM scores (shift-invariance makes the max
subtraction unnecessary), and the output DMA performs the partition
un-shift of the second half for free.
"""

import numpy as np

import concourse.bacc as bacc
import concourse.tile as tile
import concourse.mybir as mybir
from concourse import masks
from concourse.bass import _add_dep_helper
from concourse.bass_utils import run_bass_kernel_spmd

N_CORES = 8
B, DV, DT, A = 1024, 2048, 512, 512
NB = B // N_CORES          # batch rows per core (== 128 partitions)
P = 128
ND = DV // P               # 16 d-chunks for the vision projection
NT = DT // P               # 4 d-chunks for the ts projection
NA = A // P                # 4 a/j blocks
CHP = 4                    # batches per tanh op (FD = CHP*2*512 = 4096)
H2 = ND * A // 2           # free size of one Wv^T half [128, 4096]

F32 = mybir.dt.float32
F32R = mybir.dt.float32r
AF = mybir.ActivationFunctionType
ALU = mybir.AluOpType
AX = mybir.AxisListType


def build():
    nc = bacc.Bacc(
        "TRN2", target_bir_lowering=False, debug=False, num_devices=N_CORES
    )
    vis_d = nc.dram_tensor("vision_features", [NB, DV], F32, kind="ExternalInput").ap()
    ts_d = nc.dram_tensor("ts_features", [NB, DT], F32, kind="ExternalInput").ap()
    wvl_d = nc.dram_tensor("Wv_wTL", [P, H2], F32R, kind="ExternalInput").ap()
    wvr_d = nc.dram_tensor("Wv_wTR", [P, H2], F32R, kind="ExternalInput").ap()
    wvb_d = nc.dram_tensor("Wv_b", [A], F32, kind="ExternalInput").ap()
    wtc_d = nc.dram_tensor("Wt_wTc", [P, NT * A], F32R, kind="ExternalInput").ap()
    wtb_d = nc.dram_tensor("Wt_b", [A], F32, kind="ExternalInput").ap()
    vw_d = nc.dram_tensor("v_w", [A], F32, kind="ExternalInput").ap()
    out_d = nc.dram_tensor("out", [NB, DV + DT], F32, kind="ExternalOutput").ap()

    with tile.TileContext(nc) as tc:
        with (
            tc.tile_pool(name="persist", bufs=1) as persist,
            tc.tile_pool(name="scores", bufs=1, space="PSUM") as scores_pool,
            tc.tile_pool(name="s1ps", bufs=2, space="PSUM") as s1ps,
            tc.tile_pool(name="s1acc", bufs=1, space="PSUM") as s1acc,
            tc.tile_pool(name="hot", bufs=3) as hot,
        ):
            # ---------- persistent tiles ----------
            vis = persist.tile([P, DV], F32, tag="vis")
            ts_sb = persist.tile([P, DT], F32, tag="ts")
            ts_lo = persist.tile([P, DT], F32, tag="ts_lo")
            tpb = persist.tile([P, A], F32, tag="tpb")     # tp + Wt_b, [b, i]
            vpT = persist.tile([P, A], F32, tag="vpT")     # [a%128, ablk*128+b]
            vcol = persist.tile([P, NA], F32, tag="vcol")
            wvbc = persist.tile([P, NA], F32, tag="wvbc")  # Wv_b col per ablk
            wtbr = persist.tile([1, A], F32, tag="wtbr")
            wtb_bc = persist.tile([P, A], F32, tag="wtb_bc")
            ident = persist.tile([P, P], F32, tag="ident")
            # sliding-window one-hot weights: vwin[jb] is zeros except
            # column 63 = v_w[jb-block].  lhsT = vwin[jb][:, 63-r:127-r]
            # is a [128, 64] one-hot-at-r stationary operand: the matmul
            # adds v.G into row r of a 64-row PSUM tile and exact zeros
            # into the other rows.
            vwin = [
                persist.tile([P, 2 * 64], F32R, tag=f"vwin{jb}", name=f"vwin{jb}")
                for jb in range(NA)
            ]
            zwin = persist.tile([P, 2 * 64], F32, tag="zwin")
            wtT_sb = persist.tile([P, NT * A], F32R, tag="wtT_sb")
            wvl_sb = persist.tile([P, H2], F32R, tag="wvl_sb")
            wvr_sb = persist.tile([P, H2], F32R, tag="wvr_sb")
            visT = persist.tile([P, DV], F32R, tag="visT")  # [d%128, q*512+..]
            vp_sb = persist.tile([P, A], F32, tag="vp_sb")
            tsT = persist.tile([P, A], F32R, tag="tsT")

            scores_ps = [
                scores_pool.tile([P, A], F32, tag=f"scores{t}", name=f"scores{t}")
                for t in range(2)
            ]

            # ---------- input DMAs (issue order == queue order) ----------
            nc.sync.dma_start(vcol[:], vw_d.rearrange("(blk p) -> p blk", p=P))
            nc.sync.dma_start(wvbc[:], wvb_d.rearrange("(blk p) -> p blk", p=P))
            nc.sync.dma_start(wtbr[0:1, :], wtb_d[:])
            nc.sync.dma_start(ts_sb[:], ts_d[:])
            nc.sync.dma_start(wtT_sb[:], wtc_d[:])
            nc.sync.dma_start(vis[:], vis_d[:])
            for q in range(4):
                nc.sync.dma_start(
                    wvl_sb[:, q * H2 // 4:(q + 1) * H2 // 4],
                    wvl_d[:, q * H2 // 4:(q + 1) * H2 // 4],
                )

            # ---------- constants ----------
            masks.make_identity(nc, ident[:])
            nc.gpsimd.partition_broadcast(wtb_bc[:], wtbr[0:1, :])
            nc.vector.memset(zwin[:], 0.0)
            for jb in range(NA):
                nc.vector.tensor_copy(vwin[jb][:], zwin[:])
                nc.vector.tensor_copy(vwin[jb][:, 63:64], vcol[:, jb:jb + 1])

            # ---------- ts side: tp[b, a] = sum_d ts[b, d] Wt[a, d] ----
            ps = s1ps.tile([P, A], F32, tag="tr_ps", name="tr_ps")
            for dc in range(NT):
                nc.tensor.transpose(
                    ps[:, dc * P:(dc + 1) * P],
                    ts_sb[:, dc * P:(dc + 1) * P], ident[:],
                )
            nc.vector.tensor_copy(tsT[:], ps[:])
            tp_ps = s1acc.tile([P, A], F32, tag="tp_ps")
            for dc in range(NT):
                nc.tensor.matmul(
                    tp_ps[:],
                    tsT[:, dc * P:(dc + 1) * P],
                    wtT_sb[:, dc * A:(dc + 1) * A],
                    start=(dc == 0),
                    stop=(dc == NT - 1),
                )
            nc.vector.tensor_add(tpb[:], tp_ps[:], wtb_bc[:])

            # ---------- vision transposes + vp left half --------------
            for q in range(ND // NA):
                ps = s1ps.tile([P, A], F32, tag="tr_ps", name="tr_ps")
                for k in range(NA):
                    dc = q * NA + k
                    nc.tensor.transpose(
                        ps[:, k * P:(k + 1) * P],
                        vis[:, dc * P:(dc + 1) * P], ident[:],
                    )
                cp = [nc.vector.tensor_copy, nc.scalar.copy][q % 2]
                cp(visT[:, q * A:(q + 1) * A], ps[:])

            vp_ps = s1acc.tile([P, A], F32, tag="vp_ps")
            vpT_ps = s1acc.tile([P, A], F32, tag="vpT_ps")
            wvh_sb = [wvl_sb, wvr_sb]
            vpT_done = None

            def vp_half(h):
                nonlocal vpT_done
                HW = A // 2
                for dc in range(ND):
                    nc.tensor.matmul(
                        vp_ps[:, h * HW:(h + 1) * HW],
                        visT[:, dc * P:(dc + 1) * P],
                        wvh_sb[h][:, dc * HW:(dc + 1) * HW],
                        start=(dc == 0),
                        stop=(dc == ND - 1),
                    )
                nc.vector.tensor_copy(
                    vp_sb[:, h * HW:(h + 1) * HW], vp_ps[:, h * HW:(h + 1) * HW]
                )
                for ablk in (2 * h, 2 * h + 1):
                    nc.tensor.transpose(
                        vpT_ps[:, ablk * P:(ablk + 1) * P],
                        vp_sb[:, ablk * P:(ablk + 1) * P], ident[:],
                    )
                    vpT_done = nc.vector.tensor_scalar_add(
                        vpT[:, ablk * P:(ablk + 1) * P],
                        vpT_ps[:, ablk * P:(ablk + 1) * P],
                        wvbc[:, ablk:ablk + 1],
                    )

            vp_half(0)

            # ---------- the hot loop ----------------------------------
            # Hybrid schedule: batches 0..31 run as two j-phases (phase
            # 0 needs only the left Wv half, so ACT starts ~10us earlier
            # and the right half streams in underneath); batches 32..127
            # run single-pass (one broadcast serves all four j-blocks,
            # halving GPSIMD work).
            def emit_group(b0, nbat, js, starts, stops):
                rowstage = hot.tile([1, nbat * A], F32,
                                    tag="rowstage", name="rowstage", bufs=2)
                nc.sync.dma_start(rowstage[0:1, :], tpb[b0:b0 + nbat, :])
                bt = hot.tile([P, nbat * A], F32, tag="Bt", name="Bt", bufs=2)
                nc.gpsimd.partition_broadcast(bt[:], rowstage[0:1, :])
                nj = len(js)
                S = hot.tile([P, nbat * nj * A], F32, tag="S", name="S", bufs=2)
                for ci in range(nbat):
                    b = b0 + ci
                    for ki, jb in enumerate(js):
                        o = (ci * nj + ki) * A
                        nc.vector.tensor_scalar_add(
                            S[:, o:o + A],
                            bt[:, ci * A:(ci + 1) * A],
                            vpT[:, jb * P + b:jb * P + b + 1],
                        )
                G = hot.tile([P, nbat * nj * A], F32R, tag="G", name="G", bufs=2)
                nc.scalar.activation(G[:], S[:], AF.Tanh)
                for ci in range(nbat):
                    b = b0 + ci
                    t, r = divmod(b, 64)
                    for ki, jb in enumerate(js):
                        o = (ci * nj + ki) * A
                        nc.tensor.matmul(
                            scores_ps[t][0:64, :],
                            vwin[jb][:, 63 - r:127 - r],
                            G[:, o:o + A],
                            start=((b, jb) in starts),
                            stop=((b, jb) in stops),
                        )

            starts = {(0, 0), (64, 0)}
            stops = {(63, 3), (127, 3)}

            for b4 in range(8):
                emit_group(b4 * CHP, CHP, (0, 1), starts, stops)

            # right Wv half streams in while phase 0 runs
            for q in range(4):
                nc.sync.dma_start(
                    wvr_sb[:, q * H2 // 4:(q + 1) * H2 // 4],
                    wvr_d[:, q * H2 // 4:(q + 1) * H2 // 4],
                )
            nc.sync.dma_start(ts_lo[0:64, :], ts_d[64:128, :])
            vp_half(1)

            for b4 in range(8):
                emit_group(b4 * CHP, CHP, (2, 3), starts, stops)
            for b2 in range(16, 64):
                emit_group(b2 * 2, 2, (0, 1, 2, 3), starts, stops)

            # ---------- softmax + epilogue, per 64-row half ------------
            with tc.tile_pool(name="epi", bufs=1) as epi:
                for t in range(2):
                    ex = epi.tile([P, A], F32, tag=f"ex{t}", name=f"ex{t}")
                    sm = epi.tile([P, 1], F32, tag=f"sm{t}", name=f"sm{t}")
                    nc.scalar.activation(
                        ex[0:64, :], scores_ps[t][0:64, :], AF.Exp,
                        accum_out=sm[0:64, :],
                    )
                    rc = epi.tile([P, 1], F32, tag=f"rc{t}", name=f"rc{t}")
                    nc.vector.reciprocal(rc[0:64, :], sm[0:64, :])
                    aw = epi.tile([P, A], F32, tag=f"aw{t}", name=f"aw{t}")
                    nc.vector.tensor_scalar_mul(
                        aw[0:64, :], ex[0:64, :], rc[0:64, :]
                    )
                    at = epi.tile([P, A], F32, tag=f"at{t}", name=f"at{t}")
                    ts_src = ts_sb if t == 0 else ts_lo
                    nc.vector.tensor_mul(at[0:64, :], aw[0:64, :], ts_src[0:64, :])
                    nc.sync.dma_start(
                        out_d[t * 64:(t + 1) * 64, DV:DV + DT], at[0:64, :]
                    )
                for q in range(2):
                    vout = nc.sync.dma_start(
                        out_d[:, q * DV // 2:(q + 1) * DV // 2],
                        vis[:, q * DV // 2:(q + 1) * DV // 2],
                    )
                    _add_dep_helper(
                        vout.ins, vpT_done.ins, sync=False,
                        reason="defer vis passthrough behind weight loads",
                    )

    nc.compile()
    return nc


_NC_CACHE = None


def _get_nc():
    global _NC_CACHE
    if _NC_CACHE is None:
        _NC_CACHE = build()
    return _NC_CACHE


def make_in_maps(vision_features, ts_features, Wv_w, Wv_b, Wt_w, Wt_b, v_w):
    wvt = np.asarray(Wv_w, dtype=np.float32).T.reshape(ND, P, A)
    wtt = np.asarray(Wt_w, dtype=np.float32).T.reshape(NT, P, A)
    shared = {
        "Wv_wTL": np.ascontiguousarray(
            wvt[:, :, : A // 2].transpose(1, 0, 2).reshape(P, H2)
        ),
        "Wv_wTR": np.ascontiguousarray(
            wvt[:, :, A // 2:].transpose(1, 0, 2).reshape(P, H2)
        ),
        "Wv_b": np.ascontiguousarray(Wv_b, dtype=np.float32),
        "Wt_wTc": np.ascontiguousarray(
            wtt.transpose(1, 0, 2).reshape(P, NT * A)
        ),
        "Wt_b": np.ascontiguousarray(Wt_b, dtype=np.float32),
        "v_w": np.ascontiguousarray(v_w, dtype=np.float32),
    }
    in_maps = []
    for c in range(N_CORES):
        sl = slice(c * NB, (c + 1) * NB)
        in_maps.append(
            {
                "vision_features": np.ascontiguousarray(
                    vision_features[sl], dtype=np.float32
                ),
                "ts_features": np.ascontiguousarray(
                    ts_features[sl], dtype=np.float32
                ),
                **shared,
            }
        )
    return in_maps


def kernel(
    vision_features, ts_features, Wv_w, Wv_b, Wt_w, Wt_b, v_w, v_b=None, **_unused
):
    # v_b shifts every score of a row equally; softmax is invariant to it.
    nc = _get_nc()
    in_maps = make_in_maps(
        vision_features, ts_features, Wv_w, Wv_b, Wt_w, Wt_b, v_w
    )
    res = run_bass_kernel_spmd(nc, in_maps, core_ids=list(range(N_CORES)))
    return np.concatenate([res.results[c]["out"] for c in range(N_CORES)], axis=0)


# revision 31
# speedup vs baseline: 1.9942x; 1.0042x over previous
"""AdditiveAttention fused Trainium2 kernel.

Computes, for vision_features (B, DV), ts_features (B, DT) with B=1024,
DV=2048, DT=A=512:

    vp = vision @ Wv_w.T + Wv_b                     (B, A)
    tp = ts @ Wt_w.T + Wt_b                         (B, A)
    scores[b,i] = sum_j v_w[j]*tanh(tp[b,i]+vp[b,j])   (+ v_b, dropped:
                                                     softmax shift-invariant)
    attn = softmax(scores, axis=1)
    out = concat([vision, ts * attn], axis=1)       (B, DV+DT)

Data parallel over 8 NeuronCores: each core owns 128 batch rows and the
replicated (small) weights.  Per core, the B*A*A tanh tensor (the
reference's 1 GB intermediate) is built and consumed in SBUF chunks:

  DMA     stages the per-batch (tp+Wt_b) rows to partition 0 (compute
          engines can only address SBUF from partitions 0/32/64/96)
  GPSIMD  partition_broadcast of those rows           -> Bt [128, CHP*512]
  DVE     tensor_scalar add of the vp^T column (bias per j-partition)
          -> S (j on partitions, (batch, i) on the free dim)
  ACT     one FD=4096 tanh per 4-batch group — the bottleneck:
          B*A*A / 128 lanes / 1.2 GHz  ~=  218 us/core floor
  PE      fp32r matmuls against a sliding one-hot v_w window accumulate
          scores rows into two 64-row PSUM tiles (fp32r matmul output
          must start at PSUM partition 0)

The batch sweep runs in two j-phases (j-blocks {0,1} then {2,3}) so the
first tanh only needs the left half of Wv; the right half streams in
during phase 0.  Weights are pre-transposed/pre-chunked on the host so
each weight block is a single fully contiguous DMA.  Softmax is done per
64-row half directly on the PSUM scores (shift-invariance makes the max
subtraction unnecessary), and the output DMA performs the partition
un-shift of the second half for free.
"""

import numpy as np

import concourse.bacc as bacc
import concourse.tile as tile
import concourse.mybir as mybir
from concourse import masks
from concourse.bass import _add_dep_helper
from concourse.bass_utils import run_bass_kernel_spmd

N_CORES = 8
B, DV, DT, A = 1024, 2048, 512, 512
NB = B // N_CORES          # batch rows per core (== 128 partitions)
P = 128
ND = DV // P               # 16 d-chunks for the vision projection
NT = DT // P               # 4 d-chunks for the ts projection
NA = A // P                # 4 a/j blocks
CHP = 4                    # batches per tanh op (FD = CHP*2*512 = 4096)
H2 = ND * A // 2           # free size of one Wv^T half [128, 4096]

F32 = mybir.dt.float32
F32R = mybir.dt.float32r
AF = mybir.ActivationFunctionType
ALU = mybir.AluOpType
AX = mybir.AxisListType


def build():
    nc = bacc.Bacc(
        "TRN2", target_bir_lowering=False, debug=False, num_devices=N_CORES
    )
    vis_d = nc.dram_tensor("vision_features", [NB, DV], F32, kind="ExternalInput").ap()
    ts_d = nc.dram_tensor("ts_features", [NB, DT], F32, kind="ExternalInput").ap()
    wvl_d = nc.dram_tensor("Wv_wTL", [P, H2], F32R, kind="ExternalInput").ap()
    wvr_d = nc.dram_tensor("Wv_wTR", [P, H2], F32R, kind="ExternalInput").ap()
    wvb_d = nc.dram_tensor("Wv_b", [A], F32, kind="ExternalInput").ap()
    wtc_d = nc.dram_tensor("Wt_wTc", [P, NT * A], F32R, kind="ExternalInput").ap()
    wtb_d = nc.dram_tensor("Wt_b", [A], F32, kind="ExternalInput").ap()
    vw_d = nc.dram_tensor("v_w", [A], F32, kind="ExternalInput").ap()
    out_d = nc.dram_tensor("out", [NB, DV + DT], F32, kind="ExternalOutput").ap()

    with tile.TileContext(nc) as tc:
        with (
            tc.tile_pool(name="persist", bufs=1) as persist,
            tc.tile_pool(name="scores", bufs=1, space="PSUM") as scores_pool,
            tc.tile_pool(name="s1ps", bufs=2, space="PSUM") as s1ps,
            tc.tile_pool(name="s1acc", bufs=1, space="PSUM") as s1acc,
            tc.tile_pool(name="hot", bufs=3) as hot,
        ):
            # ---------- persistent tiles ----------
            vis = persist.tile([P, DV], F32, tag="vis")
            ts_sb = persist.tile([P, DT], F32, tag="ts")
            ts_lo = persist.tile([P, DT], F32, tag="ts_lo")
            tpb = persist.tile([P, A], F32, tag="tpb")     # tp + Wt_b, [b, i]
            vpT = persist.tile([P, A], F32, tag="vpT")     # [a%128, ablk*128+b]
            vcol = persist.tile([P, NA], F32, tag="vcol")
            wvbc = persist.tile([P, NA], F32, tag="wvbc")  # Wv_b col per ablk
            wtbr = persist.tile([1, A], F32, tag="wtbr")
            wtb_bc = persist.tile([P, A], F32, tag="wtb_bc")
            ident = persist.tile([P, P], F32, tag="ident")
            # sliding-window one-hot weights: vwin[jb] is zeros except
            # column 63 = v_w[jb-block].  lhsT = vwin[jb][:, 63-r:127-r]
            # is a [128, 64] one-hot-at-r stationary operand: the matmul
            # adds v.G into row r of a 64-row PSUM tile and exact zeros
            # into the other rows.
            vwin = [
                persist.tile([P, 2 * 64], F32R, tag=f"vwin{jb}", name=f"vwin{jb}")
                for jb in range(NA)
            ]
            zwin = persist.tile([P, 2 * 64], F32, tag="zwin")
            wtT_sb = persist.tile([P, NT * A], F32R, tag="wtT_sb")
            wvl_sb = persist.tile([P, H2], F32R, tag="wvl_sb")
            wvr_sb = persist.tile([P, H2], F32R, tag="wvr_sb")
            visT = persist.tile([P, DV], F32R, tag="visT")  # [d%128, q*512+..]
            vp_sb = persist.tile([P, A], F32, tag="vp_sb")
            tsT = persist.tile([P, A], F32R, tag="tsT")

            scores_ps = [
                scores_pool.tile([P, A], F32, tag=f"scores{t}", name=f"scores{t}")
                for t in range(2)
            ]

            # ---------- input DMAs (issue order == queue order) ----------
            nc.sync.dma_start(vcol[:], vw_d.rearrange("(blk p) -> p blk", p=P))
            nc.sync.dma_start(wvbc[:], wvb_d.rearrange("(blk p) -> p blk", p=P))
            nc.sync.dma_start(wtbr[0:1, :], wtb_d[:])
            nc.sync.dma_start(ts_sb[:], ts_d[:])
            nc.sync.dma_start(vis[:], vis_d[:])
            for q in range(3):
                nc.sync.dma_start(
                    wvl_sb[:, q * H2 // 4:(q + 1) * H2 // 4],
                    wvl_d[:, q * H2 // 4:(q + 1) * H2 // 4],
                )
            nc.sync.dma_start(wtT_sb[:], wtc_d[:])
            nc.sync.dma_start(
                wvl_sb[:, 3 * H2 // 4:4 * H2 // 4],
                wvl_d[:, 3 * H2 // 4:4 * H2 // 4],
            )

            # ---------- constants ----------
            masks.make_identity(nc, ident[:])
            nc.gpsimd.partition_broadcast(wtb_bc[:], wtbr[0:1, :])
            nc.vector.memset(zwin[:], 0.0)
            for jb in range(NA):
                nc.vector.tensor_copy(vwin[jb][:], zwin[:])
                nc.vector.tensor_copy(vwin[jb][:, 63:64], vcol[:, jb:jb + 1])

            # ---------- ts side: tp[b, a] = sum_d ts[b, d] Wt[a, d] ----
            ps = s1ps.tile([P, A], F32, tag="tr_ps", name="tr_ps")
            for dc in range(NT):
                nc.tensor.transpose(
                    ps[:, dc * P:(dc + 1) * P],
                    ts_sb[:, dc * P:(dc + 1) * P], ident[:],
                )
            nc.vector.tensor_copy(tsT[:], ps[:])
            tp_ps = s1acc.tile([P, A], F32, tag="tp_ps")
            for dc in range(NT):
                nc.tensor.matmul(
                    tp_ps[:],
                    tsT[:, dc * P:(dc + 1) * P],
                    wtT_sb[:, dc * A:(dc + 1) * A],
                    start=(dc == 0),
                    stop=(dc == NT - 1),
                )
            nc.vector.tensor_add(tpb[:], tp_ps[:], wtb_bc[:])

            # ---------- vision transposes + vp left half --------------
            for q in range(ND // NA):
                ps = s1ps.tile([P, A], F32, tag="tr_ps", name="tr_ps")
                for k in range(NA):
                    dc = q * NA + k
                    nc.tensor.transpose(
                        ps[:, k * P:(k + 1) * P],
                        vis[:, dc * P:(dc + 1) * P], ident[:],
                    )
                cp = [nc.vector.tensor_copy, nc.scalar.copy][q % 2]
                cp(visT[:, q * A:(q + 1) * A], ps[:])

            vp_ps = s1acc.tile([P, A], F32, tag="vp_ps")
            vpT_ps = s1acc.tile([P, A], F32, tag="vpT_ps")
            wvh_sb = [wvl_sb, wvr_sb]
            vpT_done = None

            def vp_half(h):
                nonlocal vpT_done
                HW = A // 2
                for dc in range(ND):
                    nc.tensor.matmul(
                        vp_ps[:, h * HW:(h + 1) * HW],
                        visT[:, dc * P:(dc + 1) * P],
                        wvh_sb[h][:, dc * HW:(dc + 1) * HW],
                        start=(dc == 0),
                        stop=(dc == ND - 1),
                    )
                nc.vector.tensor_copy(
                    vp_sb[:, h * HW:(h + 1) * HW], vp_ps[:, h * HW:(h + 1) * HW]
                )
                for ablk in (2 * h, 2 * h + 1):
                    nc.tensor.transpose(
                        vpT_ps[:, ablk * P:(ablk + 1) * P],
                        vp_sb[:, ablk * P:(ablk + 1) * P], ident[:],
                    )
                    vpT_done = nc.vector.tensor_scalar_add(
                        vpT[:, ablk * P:(ablk + 1) * P],
                        vpT_ps[:, ablk * P:(ablk + 1) * P],
                        wvbc[:, ablk:ablk + 1],
                    )

            vp_half(0)

            # ---------- the hot loop ----------------------------------
            # Hybrid schedule: batches 0..31 run as two j-phases (phase
            # 0 needs only the left Wv half, so ACT starts ~10us earlier
            # and the right half streams in underneath); batches 32..127
            # run single-pass (one broadcast serves all four j-blocks,
            # halving GPSIMD work).
            def emit_group(b0, nbat, js, starts, stops):
                rowstage = hot.tile([1, nbat * A], F32,
                                    tag="rowstage", name="rowstage", bufs=2)
                nc.sync.dma_start(rowstage[0:1, :], tpb[b0:b0 + nbat, :])
                bt = hot.tile([P, nbat * A], F32, tag="Bt", name="Bt", bufs=2)
                nc.gpsimd.partition_broadcast(bt[:], rowstage[0:1, :])
                nj = len(js)
                S = hot.tile([P, nbat * nj * A], F32, tag="S", name="S", bufs=2)
                for ci in range(nbat):
                    b = b0 + ci
                    for ki, jb in enumerate(js):
                        o = (ci * nj + ki) * A
                        nc.vector.tensor_scalar_add(
                            S[:, o:o + A],
                            bt[:, ci * A:(ci + 1) * A],
                            vpT[:, jb * P + b:jb * P + b + 1],
                        )
                G = hot.tile([P, nbat * nj * A], F32R, tag="G", name="G", bufs=2)
                nc.scalar.activation(G[:], S[:], AF.Tanh)
                for ci in range(nbat):
                    b = b0 + ci
                    t, r = divmod(b, 64)
                    for ki, jb in enumerate(js):
                        o = (ci * nj + ki) * A
                        nc.tensor.matmul(
                            scores_ps[t][0:64, :],
                            vwin[jb][:, 63 - r:127 - r],
                            G[:, o:o + A],
                            start=((b, jb) in starts),
                            stop=((b, jb) in stops),
                        )

            starts = {(0, 0), (64, 0)}
            stops = {(63, 3), (127, 3)}

            emit_group(0, 2, (0, 1), starts, stops)
            emit_group(2, 2, (0, 1), starts, stops)
            for b4 in range(1, 8):
                emit_group(b4 * CHP, CHP, (0, 1), starts, stops)

            # right Wv half streams in while phase 0 runs
            for q in range(4):
                nc.sync.dma_start(
                    wvr_sb[:, q * H2 // 4:(q + 1) * H2 // 4],
                    wvr_d[:, q * H2 // 4:(q + 1) * H2 // 4],
                )
            nc.sync.dma_start(ts_lo[0:64, :], ts_d[64:128, :])
            vp_half(1)

            for b4 in range(8):
                emit_group(b4 * CHP, CHP, (2, 3), starts, stops)
            for b2 in range(16, 64):
                emit_group(b2 * 2, 2, (0, 1, 2, 3), starts, stops)

            # ---------- softmax + epilogue, per 64-row half ------------
            with tc.tile_pool(name="epi", bufs=1) as epi:
                for t in range(2):
                    ex = epi.tile([P, A], F32, tag=f"ex{t}", name=f"ex{t}")
                    sm = epi.tile([P, 1], F32, tag=f"sm{t}", name=f"sm{t}")
                    nc.scalar.activation(
                        ex[0:64, :], scores_ps[t][0:64, :], AF.Exp,
                        accum_out=sm[0:64, :],
                    )
                    rc = epi.tile([P, 1], F32, tag=f"rc{t}", name=f"rc{t}")
                    nc.vector.reciprocal(rc[0:64, :], sm[0:64, :])
                    aw = epi.tile([P, A], F32, tag=f"aw{t}", name=f"aw{t}")
                    nc.vector.tensor_scalar_mul(
                        aw[0:64, :], ex[0:64, :], rc[0:64, :]
                    )
                    at = epi.tile([P, A], F32, tag=f"at{t}", name=f"at{t}")
                    ts_src = ts_sb if t == 0 else ts_lo
                    nc.vector.tensor_mul(at[0:64, :], aw[0:64, :], ts_src[0:64, :])
                    nc.sync.dma_start(
                        out_d[t * 64:(t + 1) * 64, DV:DV + DT], at[0:64, :]
                    )
                for q in range(2):
                    vout = nc.sync.dma_start(
                        out_d[:, q * DV // 2:(q + 1) * DV // 2],
                        vis[:, q * DV // 2:(q + 1) * DV // 2],
                    )
                    _add_dep_helper(
                        vout.ins, vpT_done.ins, sync=False,
                        reason="defer vis passthrough behind weight loads",
                    )

    nc.compile()
    return nc


_NC_CACHE = None


def _get_nc():
    global _NC_CACHE
    if _NC_CACHE is None:
        _NC_CACHE = build()
    return _NC_CACHE


def make_in_maps(vision_features, ts_features, Wv_w, Wv_b, Wt_w, Wt_b, v_w):
    wvt = np.asarray(Wv_w, dtype=np.float32).T.reshape(ND, P, A)
    wtt = np.asarray(Wt_w, dtype=np.float32).T.reshape(NT, P, A)
    shared = {
        "Wv_wTL": np.ascontiguousarray(
            wvt[:, :, : A // 2].transpose(1, 0, 2).reshape(P, H2)
        ),
        "Wv_wTR": np.ascontiguousarray(
            wvt[:, :, A // 2:].transpose(1, 0, 2).reshape(P, H2)
        ),
        "Wv_b": np.ascontiguousarray(Wv_b, dtype=np.float32),
        "Wt_wTc": np.ascontiguousarray(
            wtt.transpose(1, 0, 2).reshape(P, NT * A)
        ),
        "Wt_b": np.ascontiguousarray(Wt_b, dtype=np.float32),
        "v_w": np.ascontiguousarray(v_w, dtype=np.float32),
    }
    in_maps = []
    for c in range(N_CORES):
        sl = slice(c * NB, (c + 1) * NB)
        in_maps.append(
            {
                "vision_features": np.ascontiguousarray(
                    vision_features[sl], dtype=np.float32
                ),
                "ts_features": np.ascontiguousarray(
                    ts_features[sl], dtype=np.float32
                ),
                **shared,
            }
        )
    return in_maps


def kernel(
    vision_features, ts_features, Wv_w, Wv_b, Wt_w, Wt_b, v_w, v_b=None, **_unused
):
    # v_b shifts every score of a row equally; softmax is invariant to it.
    nc = _get_nc()
    in_maps = make_in_maps(
        vision_features, ts_features, Wv_w, Wv_b, Wt_w, Wt_b, v_w
    )
    res = run_bass_kernel_spmd(nc, in_maps, core_ids=list(range(N_CORES)))
    return np.concatenate([res.results[c]["out"] for c in range(N_CORES)], axis=0)
